# revision 89
# baseline (speedup 1.0000x reference)
"""Trainium2 Bass kernel for the Chebyshev atomic descriptor (gnn_message_passing).

Contract: kernel(**inputs) takes FULL unsharded inputs (positions [20000,3] f32,
species_idx [20000] i32, neighbor_idx [480000] i32) and returns the full
[20000, 52] f32 feature array. Internally shards atoms across 8 NeuronCores
(data-parallel over N), gathers neighbor data on-device via indirect DMA, and
concatenates per-core outputs on host.

v3: pair-space pipeline in fp16 on DVE (2x mode = 0.536 ns/elem), combined
A/B halving-tree reduces with per-order ring buffers, unary work on ACT,
ss + chain-B multiplies on Pool (tensor_tensor; Pool rejects TensorScalarPtr
in walrus codegen), deep io buffering so gathers pipeline ahead of compute,
and host-side angular compaction: each atom's neighbors are reordered so
angular-active ones (d <= 6.5 + margin; device masks still decide) come
first, atoms are sorted within each core by active count (descending, so the
widest supertile's compute overlaps the remaining gathers), and each
supertile's pair rectangle is sized to its own cap ka in {12,16,20,24}.
v4: gather-compaction and spin-extraction copies moved ACT->DVE (DVE idles
during the gather ramp; this unclogs ACT ahead of the critical sqrt/Sin).
Modeled per-core time 183.0us (baseline 364.2us), rel err 3.0e-3.
"""

import math
from contextlib import ExitStack

import numpy as np

import bass_rust
import concourse.bass as bass
import concourse.bacc as bacc
import concourse.tile as tile
from concourse import mybir
from concourse.bass_utils import run_bass_kernel_spmd

F32 = mybir.dt.float32
F16 = mybir.dt.float16
I32 = mybir.dt.int32
I16 = mybir.dt.int16
Alu = mybir.AluOpType
Act = mybir.ActivationFunctionType
AX = mybir.AxisListType

# ---- problem constants (hardcoded per harness contract) ----
N = 20000
K = 24
NCORES = 8
NPAD = 20480            # padded N, divisible by NCORES*128*G
NPC = NPAD // NCORES    # atoms per core = 2560
PT = 128                # partitions
G = 5                   # atoms per partition per supertile
SUP = NPC // (PT * G)   # supertiles per core = 4
KG = K * G              # neighbor slots per partition per supertile = 120
RAD_ORDER = 16
ANG_ORDER = 8
RAD_CUT = 8.0
ANG_CUT = 6.5
MIN_CUT = 0.55
DG = 12                 # circular-distance groups d=1..12
PAIR = DG * K           # 288 pair columns per atom (d=12 double-counted, half-weighted)
PAIRG = PAIR * G        # 2880
FEAT = 52
ROWE = 64               # padded table row: 64 f32 = 256B (dma_gather granularity)
GQ = 4                  # dma_gather calls per supertile
NRAD = RAD_ORDER + 1    # 17
NANG = ANG_ORDER + 1    # 9

HALF_PI = math.pi / 2.0
# x = 2*(d - MIN_CUT)/(RAD_CUT - MIN_CUT) - 1 = d*AX_ + BX_
AX_ = 2.0 / (RAD_CUT - MIN_CUT)
BX_ = -2.0 * MIN_CUT / (RAD_CUT - MIN_CUT) - 1.0

# featt column order (internal): 0..16 rad_un, 17..33 rad_w,
# 34+2t ang_un[t], 35+2t ang_w[t]  (A/B adjacent for combined tree reduce)
RAD_A0 = 0
RAD_B0 = NRAD
ANG0 = 2 * NRAD


def view(ap, off, dims):
    """Custom free-dim view of a tile AP: keep the partition entry, replace the
    free dims with explicit [step, count] pairs (supports step-0 broadcasts and
    overlapping windows), shift the in-partition element offset by `off`."""
    base = list(ap.ap[0])
    return bass_rust.AP(ap.tensor, ap.offset + off, [base] + [list(d) for d in dims])


def build_supertile(nc, io, kp, app, s, pos4, idx16, pself, feat, half_pi, ka):
    base = s * PT * G  # first atom (core-local) of this supertile
    # angular compaction: only the first ka (<= K) reordered neighbor slots
    # participate in the pair rectangle (host puts angular-active ones first)
    dg = ka // 2
    pair = dg * ka
    pairg = pair * G
    hp = pair // 2

    # ---- loads ----
    ps = io.tile([PT, 4 * G], F32, tag="ps")
    nc.sync.dma_start(
        out=ps[:],
        in_=pself[base : base + PT * G, :].rearrange("(p g) c -> p (g c)", p=PT),
    )
    # gather neighbor [x,y,z,s] rows via dma_gather (256B table rows), then
    # compact the leading 16B of each gathered row into pn [PT, KG*4]
    pn = io.tile([PT, KG * 4], F32, tag="pn")
    CQ = KG // GQ  # gathered (g,k) chunks per dma_gather call
    NIDX = CQ * PT
    for q in range(GQ):
        idx_t = io.tile([PT, NIDX // 16], I16, tag="idx_t")
        row0 = (s * GQ + q) * PT
        nc.sync.dma_start(out=idx_t[:], in_=idx16[row0 : row0 + PT, :])
        pnw = io.tile([PT, CQ * ROWE], F32, tag="pnw")
        nc.gpsimd.dma_gather(
            out_ap=view(pnw[:], 0, [[ROWE, CQ], [1, ROWE]]),
            in_ap=pos4,
            idxs_ap=idx_t[:],
            num_idxs=NIDX,
            num_idxs_reg=NIDX,
            elem_size=ROWE,
            single_packet=False,
        )
        nc.vector.tensor_copy(
            view(pn[:], q * CQ * 4, [[1, CQ * 4]]),
            view(pnw[:], 0, [[ROWE, CQ], [1, 4]]),
        )

    # ---- K-space prep (free dims (G, K) = 240 elems) ----
    # rvec = pos_nbr - pos_self  (f32)
    r_c = []
    for c in range(3):
        r = kp.tile([PT, KG], F32, tag=f"r{c}")
        nc.vector.tensor_tensor(
            out=r[:].rearrange("p (g k) -> p g k", g=G),
            in0=view(pn[:], c, [[4 * K, G], [4, K]]),
            in1=view(ps[:], c, [[4, G], [0, K]]),
            op=Alu.subtract,
        )
        r_c.append(r)
    # d2 = rx^2 + ry^2 + rz^2  (squares on ACT, adds on DVE)
    sq = []
    for c in range(3):
        q = kp.tile([PT, KG], F32, tag=f"sq{c}")
        nc.scalar.activation(q[:], r_c[c][:], Act.Square)
        sq.append(q)
    d2 = kp.tile([PT, KG], F32, tag="d2")
    nc.vector.tensor_tensor(out=d2[:], in0=sq[0][:], in1=sq[1][:], op=Alu.add)
    nc.vector.tensor_tensor(out=d2[:], in0=d2[:], in1=sq[2][:], op=Alu.add)
    # clamp to avoid rsqrt(0); masked-out anyway (d <= MIN_CUT)
    nc.vector.tensor_scalar_max(d2[:], d2[:], 1e-18)
    dd = kp.tile([PT, KG], F32, tag="dd")
    nc.scalar.sqrt(dd[:], d2[:])
    rinv = kp.tile([PT, KG], F32, tag="rinv")
    nc.vector.reciprocal(rinv[:], dd[:])

    # unit vectors (f16) into extended (wrap-around) buffers [G, 36]
    ue = []
    for c in range(3):
        e = kp.tile([PT, 36 * G], F16, tag=f"ue{c}")
        nc.vector.tensor_tensor(
            out=view(e[:], 0, [[36, G], [1, K]]),
            in0=r_c[c][:].rearrange("p (g k) -> p g k", g=G),
            in1=rinv[:].rearrange("p (g k) -> p g k", g=G),
            op=Alu.mult,
        )
        ue.append(e)

    # masks: m2 = (d > MIN_CUT); m1h = 0.5*(d <= RAD_CUT); a1h = 0.5*(d <= ANG_CUT)
    m2 = kp.tile([PT, KG], F32, tag="m2")
    nc.vector.tensor_scalar(
        out=m2[:], in0=dd[:], scalar1=MIN_CUT, scalar2=None, op0=Alu.is_gt
    )
    m1h = kp.tile([PT, KG], F32, tag="m1h")
    nc.vector.tensor_scalar(
        out=m1h[:], in0=dd[:], scalar1=RAD_CUT, scalar2=0.5, op0=Alu.is_le, op1=Alu.mult
    )
    a1h = kp.tile([PT, KG], F32, tag="a1h")
    nc.vector.tensor_scalar(
        out=a1h[:], in0=dd[:], scalar1=ANG_CUT, scalar2=0.5, op0=Alu.is_le, op1=Alu.mult
    )
    # cos cutoffs via sin(pi/2 - pi*min(d,rc)/rc) = cos(pi*d/rc) for in-mask d.
    dcr = kp.tile([PT, KG], F32, tag="dcr")
    nc.vector.tensor_scalar_min(dcr[:], dd[:], RAD_CUT)
    grad = kp.tile([PT, KG], F32, tag="grad")
    nc.scalar.activation(
        grad[:], dcr[:], Act.Sin, bias=half_pi[:], scale=-math.pi / RAD_CUT
    )
    dca = kp.tile([PT, KG], F32, tag="dca")
    nc.vector.tensor_scalar_min(dca[:], dd[:], ANG_CUT)
    gang = kp.tile([PT, KG], F32, tag="gang")
    nc.scalar.activation(
        gang[:], dca[:], Act.Sin, bias=half_pi[:], scale=-math.pi / ANG_CUT
    )

    # radial ring slots [S_t | sn*S_t] (A/B halves, combined reduce per order)
    rbuf = [kp.tile([PT, 2 * KG], F16, name=f"Sr{i}", tag=f"Sr{i}") for i in range(3)]

    # radial weights wr = fc*m = mh*(grad+1) -> f16 seed, directly into rbuf[0]
    mh = kp.tile([PT, KG], F32, tag="mh")
    nc.vector.tensor_tensor(out=mh[:], in0=m1h[:], in1=m2[:], op=Alu.mult)
    mgr = kp.tile([PT, KG], F32, tag="mgr")
    nc.vector.tensor_tensor(out=mgr[:], in0=mh[:], in1=grad[:], op=Alu.mult)
    nc.vector.tensor_tensor(
        out=view(rbuf[0][:], 0, [[1, KG]]), in0=mgr[:], in1=mh[:], op=Alu.add
    )

    # angular per-neighbor weights fcm = fca*m_ang (f16 ext); neighbor spin (f16 ext)
    fcme = kp.tile([PT, 36 * G], F16, tag="fcme")
    se = kp.tile([PT, 36 * G], F16, tag="se")
    fcm_b = view(fcme[:], 0, [[36, G], [1, K]])
    mA = kp.tile([PT, KG], F32, tag="mA")
    nc.vector.tensor_tensor(out=mA[:], in0=a1h[:], in1=m2[:], op=Alu.mult)
    mga = kp.tile([PT, KG], F32, tag="mga")
    nc.vector.tensor_tensor(out=mga[:], in0=mA[:], in1=gang[:], op=Alu.mult)
    mA3 = mA[:].rearrange("p (g k) -> p g k", g=G)
    nc.vector.tensor_tensor(
        out=fcm_b, in0=mga[:].rearrange("p (g k) -> p g k", g=G), in1=mA3, op=Alu.add
    )
    # neighbor typespin -> f16: se ext for angular (first ka slots used),
    # sn16 separately for radial (se's wrap copy clobbers slots >= ka)
    nc.vector.tensor_copy(
        view(se[:], 0, [[36, G], [1, K]]),
        view(pn[:], 3, [[4 * K, G], [4, K]]),
    )
    sn16 = kp.tile([PT, KG], F16, tag="sn16")
    nc.vector.tensor_copy(
        sn16[:].rearrange("p (g k) -> p g k", g=G),
        view(pn[:], 3, [[4 * K, G], [4, K]]),
    )

    # wrap-around copies: ext[:, ka:ka+dg] = ext[:, 0:dg]  (f16)
    for e in (*ue, fcme, se):
        nc.vector.tensor_copy(
            view(e[:], ka, [[36, G], [1, dg]]), view(e[:], 0, [[36, G], [1, dg]])
        )

    # x map and 2x (f16, from f32 dd on ACT)
    xx = kp.tile([PT, KG], F16, tag="xx")
    nc.scalar.activation(xx[:], dd[:], Act.Copy, bias=BX_, scale=AX_)
    x2 = kp.tile([PT, KG], F16, tag="x2")
    nc.scalar.activation(x2[:], dd[:], Act.Copy, bias=2 * BX_, scale=2 * AX_)

    # feature accumulator (f32)
    featt = app.tile([PT, G * FEAT], F32, tag="featt")

    # ================= radial chains (f16) =================
    # S_t = wr*T_t(x) in the A-half of a 2-half ring slot; B = sn*S_t in the
    # B-half; one combined segmented reduce per order -> featt cols (t, 17+t).
    rtmpa = kp.tile([PT, KG], F16, tag="rtmpa")
    sn_v = view(sn16[:], 0, [[K, G], [1, K]])  # f16 spin [G,K]

    def rSv(t):
        return view(rbuf[t % 3][:], 0, [[1, KG]])

    def emit_radial_order(t):
        if t == 0:
            pass  # seed already in rbuf[0]
        elif t == 1:
            nc.vector.tensor_tensor(out=rSv(1), in0=xx[:], in1=rSv(0), op=Alu.mult)
        else:
            nc.vector.tensor_tensor(out=rtmpa[:], in0=x2[:], in1=rSv(t - 1), op=Alu.mult)
            nc.vector.tensor_tensor(
                out=rSv(t), in0=rtmpa[:], in1=rSv(t - 2), op=Alu.subtract
            )
        r = rbuf[t % 3]
        nc.vector.tensor_tensor(
            out=view(r[:], KG, [[K, G], [1, K]]),
            in0=view(r[:], 0, [[K, G], [1, K]]),
            in1=sn_v,
            op=Alu.mult,
        )
        nc.vector.tensor_reduce(
            out=view(featt[:], RAD_A0 + t, [[NRAD, 2], [FEAT, G]]),
            in_=view(r[:], 0, [[KG, 2], [K, G], [1, K]]),
            axis=AX.X,
            op=Alu.add,
        )

    # ================= angular (f16) =================
    # pair rectangle (g, d=1..12, i=0..23); value rings hold [S_t | B_t]
    # adjacent so one halving tree reduces both chains.
    ring = [app.tile([PT, 2 * PAIRG], F16, name=f"ring{i}", tag=f"ring{i}") for i in range(4)]
    scr = app.tile([PT, PAIRG], F16, tag="scr")
    atmp = app.tile([PT, PAIRG], F16, tag="atmp")
    ss = app.tile([PT, PAIRG], F16, tag="ss")
    ct = app.tile([PT, PAIRG], F16, tag="ct")
    c2t = app.tile([PT, PAIRG], F16, tag="c2t")

    def Sv(t):   # chain-A half of ring slot for order t
        return view(ring[t % 4][:], 0, [[1, pairg]])

    def Bv(t):   # chain-B half (fixed PAIRG offset, pairg used width)
        return view(ring[t % 4][:], PAIRG, [[1, pairg]])

    # pair sign ss on Pool first: its only input is se (wraps done), so it
    # overlaps the DVE ct build below
    nc.gpsimd.tensor_tensor(
        out=view(ss[:], 0, [[pair, G], [ka, dg], [1, ka]]),
        in0=view(se[:], 0, [[36, G], [0, dg], [1, ka]]),
        in1=view(se[:], 1, [[36, G], [1, dg], [1, ka]]),
        op=Alu.mult,
    )

    # cos(theta): ct = sum_c u[jj]*u[kk] over the pair rectangle
    ct3 = view(ct[:], 0, [[pair, G], [ka, dg], [1, ka]])
    tp3 = view(atmp[:], 0, [[pair, G], [ka, dg], [1, ka]])
    ctf = view(ct[:], 0, [[1, pairg]])
    tpf = view(atmp[:], 0, [[1, pairg]])
    for c in range(3):
        jj = view(ue[c][:], 0, [[36, G], [0, dg], [1, ka]])
        kk = view(ue[c][:], 1, [[36, G], [1, dg], [1, ka]])
        if c == 0:
            nc.vector.tensor_tensor(out=ct3, in0=jj, in1=kk, op=Alu.mult)
        else:
            nc.vector.tensor_tensor(out=tp3, in0=jj, in1=kk, op=Alu.mult)
            nc.vector.tensor_tensor(out=ctf, in0=ctf, in1=tpf, op=Alu.add)
    # 2*ct on DVE (f16 tensor_scalar runs in 4x mode)
    nc.vector.tensor_scalar(
        out=view(c2t[:], 0, [[1, pairg]]), in0=ctf, scalar1=2.0, scalar2=None,
        op0=Alu.mult,
    )

    # pair weight w = fcm_j*fcm_k (d=dg halved) -> S_0
    nc.vector.tensor_tensor(
        out=view(ring[0][:], 0, [[pair, G], [ka, dg], [1, ka]]),
        in0=view(fcme[:], 0, [[36, G], [0, dg], [1, ka]]),
        in1=view(fcme[:], 1, [[36, G], [1, dg], [1, ka]]),
        op=Alu.mult,
    )
    dv = view(ring[0][:], (dg - 1) * ka, [[pair, G], [1, ka]])
    nc.vector.tensor_scalar(out=dv, in0=dv, scalar1=0.5, scalar2=None, op0=Alu.mult)

    def tree_reduce(t):
        """Combined A/B halving tree (within each atom's pair block):
        ring slot [S_t | B_t] -> featt cols (34+t, 43+t)."""
        r = ring[t % 4]
        # L1: [atom][pair] -> scr [atom][hp], per chain (split so the A half
        # does not wait on Pool's B_t)
        for ch in range(2):
            nc.vector.tensor_tensor(
                out=view(scr[:], ch * hp * G, [[hp, G], [1, hp]]),
                in0=view(r[:], ch * PAIRG, [[pair, G], [1, hp]]),
                in1=view(r[:], ch * PAIRG + hp, [[pair, G], [1, hp]]),
                op=Alu.add,
            )
        n = hp
        while n > 18 and n % 2 == 0:  # halve while clean (stops at odd/<=18)
            nc.vector.tensor_tensor(
                out=view(scr[:], 0, [[hp * G, 2], [hp, G], [1, n // 2]]),
                in0=view(scr[:], 0, [[hp * G, 2], [hp, G], [1, n // 2]]),
                in1=view(scr[:], n // 2, [[hp * G, 2], [hp, G], [1, n // 2]]),
                op=Alu.add,
            )
            n //= 2
        nc.vector.tensor_reduce(
            out=view(featt[:], ANG0 + t, [[NANG, 2], [FEAT, G]]),
            in_=view(scr[:], 0, [[hp * G, 2], [hp, G], [1, n]]),
            axis=AX.X,
            op=Alu.add,
        )

    def chain_b(t, on_pool):
        src, dst = Sv(t), Bv(t)
        eng = nc.gpsimd if on_pool else nc.vector
        eng.tensor_tensor(out=dst, in0=src, in1=view(ss[:], 0, [[1, pairg]]), op=Alu.mult)

    # Interleave: angular order t (rec on DVE, B on Pool), then two radial
    # orders (fills DVE/Pool gaps while Pool computes B_t), then the tree of
    # the PREVIOUS angular order (so it never head-of-line blocks on Pool).
    rad_next = 0

    def emit_radial(n):
        nonlocal rad_next
        for _ in range(n):
            if rad_next < NRAD:
                emit_radial_order(rad_next)
                rad_next += 1

    # t=0: S_0 = w (already in ring0)
    chain_b(0, on_pool=True)
    emit_radial(2)
    # t=1: S_1 = ct*w
    nc.vector.tensor_tensor(out=Sv(1), in0=ctf, in1=Sv(0), op=Alu.mult)
    chain_b(1, on_pool=True)
    emit_radial(2)
    for t in range(2, NANG):
        nc.vector.tensor_tensor(out=tpf, in0=view(c2t[:], 0, [[1, pairg]]), in1=Sv(t - 1), op=Alu.mult)
        nc.vector.tensor_tensor(out=Sv(t), in0=tpf, in1=Sv(t - 2), op=Alu.subtract)
        chain_b(t, on_pool=True)
        emit_radial(2)
        if t >= 2:
            tree_reduce(t - 2)  # lag hides Pool's B latency
    emit_radial(NRAD)
    for t in range(NANG - 2, NANG):
        tree_reduce(t)

    # ---- store (featt is already in final column order) ----
    nc.sync.dma_start(
        out=feat[base : base + PT * G, :].rearrange("(p g) f -> p (g f)", p=PT),
        in_=featt[:],
    )


def build_program(kas, debug=False):
    """kas: per-supertile angular widths (tuple of SUP values from KA_MENU)."""
    nc = bacc.Bacc(
        "TRN2",
        target_bir_lowering=False,
        debug=False,
        dynamic_dma_scratch_size=16384,
    )
    pos4 = nc.dram_tensor("pos4", [NPAD, ROWE], F32, kind="ExternalInput").ap()
    idx16 = nc.dram_tensor(
        "idx16", [SUP * GQ * PT, KG // GQ * PT // 16], I16, kind="ExternalInput"
    ).ap()
    pself = nc.dram_tensor("pself", [NPC, 4], F32, kind="ExternalInput").ap()
    feat = nc.dram_tensor("feat", [NPC, FEAT], F32, kind="ExternalOutput").ap()
    with tile.TileContext(nc) as tc, ExitStack() as ctx:
        io = ctx.enter_context(tc.tile_pool(name="io", bufs=6))
        kp = ctx.enter_context(tc.tile_pool(name="kspace", bufs=2))
        app = ctx.enter_context(tc.tile_pool(name="pairspace", bufs=1))
        const = ctx.enter_context(tc.tile_pool(name="const", bufs=1))
        half_pi = const.tile([PT, 1], F32, tag="half_pi")
        nc.gpsimd.memset(half_pi[:], HALF_PI)
        for s in range(SUP):
            build_supertile(
                nc, io, kp, app, s, pos4, idx16, pself, feat, half_pi, kas[s]
            )
    nc.compile()
    return nc


KA_MENU = (12, 16, 20, 24)
_NC_CACHE = {}
_NC_LAST = None


def get_program(kas=None):
    """kas=None returns the most recently built program (the one kernel()/run()
    actually executed); first-ever call defaults to the uncompacted widths."""
    global _NC_LAST
    if kas is None:
        if _NC_LAST is None:
            _NC_LAST = (K,) * SUP
        kas = _NC_LAST
    kas = tuple(kas)
    if kas not in _NC_CACHE:
        _NC_CACHE[kas] = build_program(kas)
    _NC_LAST = kas
    return _NC_CACHE[kas]


def make_in_maps(positions, species_idx, neighbor_idx):
    """Host-side layout: reorder each atom's neighbors so angular-active ones
    (d <= ANG_CUT + margin) come first, sort each core's atoms by active count,
    and pick per-supertile pair widths. The device still computes all masks, so
    correctness only needs host-active to be a superset of device-active.
    Returns (in_maps, perms, kas)."""
    pos4 = np.zeros((NPAD, ROWE), np.float32)
    pos4[:N, :3] = positions
    pos4[:N, 3] = 2.0 * species_idx.astype(np.float32) - 1.0  # TYPESPIN[-1, 1]
    nbrK = np.zeros((NPAD, K), np.int32)
    nbrK[:N] = neighbor_idx.reshape(N, K)

    # angular-active flags with margin (pad atoms: none active)
    pos = pos4[:NPAD, :3]
    rv = pos[nbrK] - pos[:NPAD, None, :]
    d2h = np.einsum("nkc,nkc->nk", rv, rv)
    active = d2h <= (ANG_CUT + 1e-3) ** 2
    active[N:] = False
    order = np.argsort(~active, axis=1, kind="stable")  # actives first
    nbrK = np.take_along_axis(nbrK, order, axis=1)
    n_act = active.sum(axis=1)

    CQ = KG // GQ
    c_idx = np.arange(KG)
    g_of, k_of = c_idx // K, c_idx % K
    p = np.arange(PT)
    in_maps, perms = [], []
    sup_max = np.zeros(SUP, np.int64)
    for c in range(NCORES):
        cb = c * NPC
        # descending: widest (most expensive) supertile first so its compute
        # overlaps the remaining supertiles' gathers
        perm = np.argsort(-n_act[cb : cb + NPC], kind="stable")
        perms.append(perm)
        for s in range(SUP):
            sup_max[s] = max(
                sup_max[s], n_act[cb + perm[s * PT * G : (s + 1) * PT * G]].max()
            )
    kas = tuple(min(m for m in KA_MENU if m >= mx) for mx in sup_max)

    for c in range(NCORES):
        cb = c * NPC
        perm = perms[c]
        blocks = []
        for s in range(SUP):
            # atom at (s, partition p, sub-row g) = cb + perm[s*PT*G + p*G + g]
            loc = s * PT * G + p[None, :] * G + g_of[:, None]  # [KG, PT]
            atoms = cb + perm[loc]
            vals = nbrK[atoms, k_of[:, None]].astype(np.int16)  # [KG, PT]
            for q in range(GQ):
                flat = vals[q * CQ : (q + 1) * CQ, :].reshape(-1)  # i = cc*128+p
                wrapped = flat.reshape(-1, 16).T  # [16, NIDX/16]
                blocks.append(np.tile(wrapped, (PT // 16, 1)))
        idx16 = np.concatenate(blocks, axis=0)  # [SUP*GQ*PT, NIDX/16]
        in_maps.append(
            {
                "pos4": pos4,
                "idx16": np.ascontiguousarray(idx16),
                "pself": np.ascontiguousarray(pos4[cb + perm, :4]),
            }
        )
    return in_maps, perms, kas


def run(positions, species_idx, neighbor_idx, trace=False, trace_cores=None):
    in_maps, perms, kas = make_in_maps(positions, species_idx, neighbor_idx)
    nc = get_program(kas)
    res = run_bass_kernel_spmd(
        nc,
        in_maps,
        core_ids=list(range(NCORES)),
        trace=trace,
        trace_cores=trace_cores,
    )
    out = np.empty((NPAD, FEAT), np.float32)
    for c in range(NCORES):
        out[c * NPC + perms[c]] = res.results[c]["feat"]
    return out[:N], res


def kernel(positions, species_idx, neighbor_idx):
    out, _ = run(positions, species_idx, neighbor_idx, trace=False)
    return out
